# revision 1
# baseline (speedup 1.0000x reference)
"""LightGCN (3-layer) propagation kernel for 8 TRN2 NeuronCores.

Strategy (hardcoded for N=150000 nodes, D=64, E=1250000 edges, 3 layers):
  - Normalization factorizes: norm(e) = f[src]*f[dst] with f = deg^-1/2
    (0 for deg=0). Work in g-space: g = f*emb; per layer h[dst] += g[src];
    cur = f*h; next g = f2*h; acc += cur. out = (emb + sum cur_l)/4.
  - Nodes sharded 8 ways by id (18750/shard, padded to 18816=128*147).
    Edges partitioned by dst shard -> per-core token stream.
  - Per layer, per core: indirect-DMA gather (128 tokens/call, one row
    index per partition) from the replicated g table, then indirect-DMA
    scatter-accumulate into one of 4 rotating h accumulators. Host deals
    dst-sorted tokens round-robin across calls so each call's 128 dst
    rows are distinct (no same-call RMW races); same-dst tokens in
    different calls land in different accumulators or are serialized.
  - Between layers: rescale h -> g shard, AllGather shards into the next
    replicated g table.
"""
import sys, types, os

sys.path.insert(0, "/opt/trn_rl_repo")
import numpy as np

# ---- environment shims (missing antenv.axon_hooks module in this image) ----
if "antenv.axon_hooks" not in sys.modules:
    _m = types.ModuleType("antenv.axon_hooks")
    _h = [None]
    _m.set_axon_ntff_profile_hook = lambda v: _h.__setitem__(0, v)
    _m.get_axon_ntff_profile_hook = lambda: _h[0]
    sys.modules["antenv.axon_hooks"] = _m
    try:
        import antenv
        antenv.axon_hooks = _m
        from trn_agent_boot.trn_boot import _ntff_profile_via_ctypes
        _m.set_axon_ntff_profile_hook(
            _ntff_profile_via_ctypes("/opt/axon/libaxon_pjrt.so"))
    except Exception:
        pass

from concourse import bass, tile
from concourse import bass_utils
from concourse.bass import mybir, IndirectOffsetOnAxis
from concourse.bass_types import AP
from concourse.bacc import Bacc

bass_utils.upload_artifacts = lambda tmpdir: "local://" + tmpdir
_orig_walrus_args = bass_utils.get_walrus_args
def _walrus_args(*a, **k):
    r = _orig_walrus_args(*a, **k)
    flag = "--dge-levels=vector_dynamic_offsets,dst_reduce"
    if flag not in r:
        r = r + [flag]
    return r
bass_utils.get_walrus_args = _walrus_args

# ---- problem constants ----
NUM_USERS = 100000
NUM_ITEMS = 50000
N = NUM_USERS + NUM_ITEMS          # 150000
D = 64
L = 3                              # propagation layers
W = 8                              # cores
NS = N // W                        # 18750 real rows per shard
SH = 18816                         # padded shard rows = 128*147
COLS = SH // 128                   # 147
GROWS = W * SH                     # 150528 padded global rows
PAD_G = SH - 2                     # a guaranteed-zero g row (local), 18814
PAD_H = SH - 1                     # scrap h row (local), 18815
NACC = 4                           # rotating h accumulators

_cache = {}


def _pad_local(v):
    """global node id -> padded global row id"""
    s = v // NS
    return s * SH + (v - s * NS)


def _build_plan(edge_index):
    """Host-side preprocessing: per-core call-table of gather/scatter
    indices, plus per-node scale vectors."""
    src = np.asarray(edge_index[0], dtype=np.int64)
    dst = np.asarray(edge_index[1], dtype=np.int64)
    E = src.shape[0]

    deg = np.bincount(dst, minlength=N).astype(np.float64)
    f = np.where(deg > 0, deg ** -0.5, 0.0).astype(np.float32)

    core_of = dst // NS
    g_idx_cols = []
    s_idx_cols = []
    ncall_per_core = []
    for c in range(W):
        m = core_of == c
        sc, dc = src[m], dst[m] - c * NS
        order = np.argsort(dc, kind="stable")
        sc, dc = sc[order], dc[order]
        k = sc.shape[0]
        ncalls = (k + 127) // 128
        kp = ncalls * 128
        # deal tokens column-major into [ncalls, 128]: call j gets tokens
        # j, j+ncalls, ... -> consecutive same-dst sorted tokens spread
        # across different calls.
        gfull = np.full(kp, PAD_G, dtype=np.int64)       # local-of-block-0 pad
        sfull = np.full(kp, PAD_H, dtype=np.int64)
        gfull[:k] = _pad_local(sc)
        sfull[:k] = dc
        gmat = gfull.reshape(128, ncalls)   # token (p, j) = sorted pos j*128+p?
        smat = sfull.reshape(128, ncalls)
        # reshape(128, ncalls) row-major: element (p, j) = flat p*ncalls + j
        # we want call j slot p = sorted token j + p*ncalls -> exactly that.
        g_idx_cols.append(gmat.astype(np.int32))
        s_idx_cols.append(smat.astype(np.int32))
        ncall_per_core.append(ncalls)

    ncalls = max(ncall_per_core)
    for c in range(W):
        nc_ = g_idx_cols[c].shape[1]
        if nc_ < ncalls:
            gp = np.full((128, ncalls - nc_), PAD_G, np.int32)
            sp = np.full((128, ncalls - nc_), PAD_H, np.int32)
            g_idx_cols[c] = np.concatenate([g_idx_cols[c], gp], axis=1)
            s_idx_cols[c] = np.concatenate([s_idx_cols[c], sp], axis=1)

    return f, g_idx_cols, s_idx_cols, ncalls


def _pack_rows(a_shard):
    """[SH, 64] -> [128, COLS*64] partition-major (row r -> (p=r//COLS, r%COLS))"""
    return a_shard.reshape(128, COLS * D)


def _build_bass(ncalls):
    nc = Bacc(None)
    dt = mybir.dt
    gtab0 = nc.dram_tensor("gtab0", [GROWS, D], dt.float32, kind="ExternalInput")
    gidx = nc.dram_tensor("gidx", [128, ncalls], dt.int32, kind="ExternalInput")
    sidx = nc.dram_tensor("sidx", [128, ncalls], dt.int32, kind="ExternalInput")
    fsc = nc.dram_tensor("fsc", [128, COLS], dt.float32, kind="ExternalInput")
    f2sc = nc.dram_tensor("f2sc", [128, COLS], dt.float32, kind="ExternalInput")
    embp = nc.dram_tensor("embp", [128, COLS * D], dt.float32, kind="ExternalInput")
    out = nc.dram_tensor("out", [128, COLS * D], dt.float32, kind="ExternalOutput")

    hs = [nc.dram_tensor(f"hacc{r}", [SH, D], dt.float32) for r in range(NACC)]
    gsh = nc.dram_tensor("gsh", [SH, D], dt.float32)
    gtabA = nc.dram_tensor("gtabA", [GROWS, D], dt.float32, addr_space="Shared")
    gtabB = nc.dram_tensor("gtabB", [GROWS, D], dt.float32, addr_space="Shared")
    gtabs = [gtab0, gtabA, gtabB]

    with tile.TileContext(nc) as tc:
        with (
            tc.tile_pool(name="persist", bufs=1) as pp,
            tc.tile_pool(name="big", bufs=1) as bp,
            tc.tile_pool(name="msg", bufs=12) as mp,
        ):
            gi = pp.tile([128, ncalls], dt.int32, tag="gi")
            si = pp.tile([128, ncalls], dt.int32, tag="si")
            fs = pp.tile([128, COLS], dt.float32, tag="fs")
            f2s = pp.tile([128, COLS], dt.float32, tag="f2s")
            zt = pp.tile([128, 1176], dt.float32, tag="zt")
            acc = bp.tile([128, COLS * D], dt.float32, tag="acc")
            ht = bp.tile([128, COLS * D], dt.float32, tag="ht")
            tmp = bp.tile([128, COLS * D], dt.float32, tag="tmp")

            nc.sync.dma_start(out=gi[:], in_=gidx[:])
            nc.sync.dma_start(out=si[:], in_=sidx[:])
            nc.sync.dma_start(out=fs[:], in_=fsc[:])
            nc.sync.dma_start(out=f2s[:], in_=f2sc[:])
            nc.sync.dma_start(out=acc[:], in_=embp[:])
            nc.vector.memset(zt[:], 0.0)

            def f_bcast(t):
                ap = t[:]
                return AP(ap.tensor, ap.offset, [ap.ap[0], ap.ap[1], [0, D]])

            for layer in range(L):
                gt = gtabs[layer]
                # zero the accumulators (4 x 4.8MB, from the zero tile)
                zsz = 128 * 1176
                hflat_n = SH * D
                for r in range(NACC):
                    hv = hs[r][:, :]
                    flat = AP(hv.tensor, 0, [[1, hflat_n]])
                    for q in range(hflat_n // zsz):
                        seg = AP(hv.tensor, q * zsz, [[1176, 128], [1, 1176]])
                        nc.sync.dma_start(out=seg, in_=zt[:])

                for j in range(ncalls):
                    m = mp.tile([128, D], dt.float32, tag="m")
                    nc.gpsimd.indirect_dma_start(
                        out=m[:],
                        out_offset=None,
                        in_=gt[:, :],
                        in_offset=IndirectOffsetOnAxis(ap=gi[:, j:j + 1], axis=0),
                    )
                    nc.gpsimd.indirect_dma_start(
                        out=hs[j % NACC][:, :],
                        out_offset=IndirectOffsetOnAxis(ap=si[:, j:j + 1], axis=0),
                        in_=m[:],
                        in_offset=None,
                        compute_op=mybir.AluOpType.add,
                    )

                # ht = sum of accumulators
                nc.sync.dma_start(out=ht[:], in_=AP(hs[0][:, :].tensor, 0,
                                                    [[COLS * D, 128], [1, COLS * D]]))
                for r in range(1, NACC):
                    nc.sync.dma_start(out=tmp[:], in_=AP(hs[r][:, :].tensor, 0,
                                                         [[COLS * D, 128], [1, COLS * D]]))
                    nc.vector.tensor_add(ht[:], ht[:], tmp[:])

                ht3 = ht[:].rearrange("p (c d) -> p c d", d=D)
                tmp3 = tmp[:].rearrange("p (c d) -> p c d", d=D)
                acc3 = acc[:].rearrange("p (c d) -> p c d", d=D)
                # cur = f * h ; acc += cur
                nc.vector.tensor_tensor(out=tmp3, in0=ht3, in1=f_bcast(fs),
                                        op=mybir.AluOpType.mult)
                nc.vector.tensor_add(acc3, acc3, tmp3)
                if layer < L - 1:
                    # g_next = f2 * h ; publish shard and all-gather
                    nc.vector.tensor_tensor(out=ht3, in0=ht3, in1=f_bcast(f2s),
                                            op=mybir.AluOpType.mult)
                    nc.sync.dma_start(
                        out=AP(gsh[:, :].tensor, 0, [[COLS * D, 128], [1, COLS * D]]),
                        in_=ht[:])
                    nc.gpsimd.collective_compute(
                        "AllGather",
                        mybir.AluOpType.bypass,
                        replica_groups=[list(range(W))],
                        ins=[gsh[:, :]],
                        outs=[gtabs[layer + 1][:, :]],
                    )

            nc.vector.tensor_scalar_mul(acc[:], acc[:], 0.25)
            nc.sync.dma_start(out=out[:, :], in_=acc[:])
    nc.finalize()
    return nc


def kernel(edge_index, user_weight, item_weight):
    edge_index = np.asarray(edge_index)
    user_weight = np.asarray(user_weight, dtype=np.float32)
    item_weight = np.asarray(item_weight, dtype=np.float32)

    f, g_idx, s_idx, ncalls = _build_plan(edge_index)
    emb = np.concatenate([user_weight, item_weight], axis=0)  # [N, D]

    # padded tables
    g0 = np.zeros((GROWS, D), np.float32)
    emb_pad = np.zeros((W, SH, D), np.float32)
    f_pad = np.zeros((W, SH), np.float32)
    for s in range(W):
        blk = emb[s * NS:(s + 1) * NS]
        emb_pad[s, :NS] = blk
        f_pad[s, :NS] = f[s * NS:(s + 1) * NS]
        g0[s * SH:s * SH + NS] = blk * f_pad[s, :NS, None]

    key = ncalls
    if key not in _cache:
        _cache[key] = _build_bass(ncalls)
    nc = _cache[key]

    in_maps = []
    for c in range(W):
        fp = f_pad[c].reshape(128, COLS)
        in_maps.append({
            "gtab0": g0,
            "gidx": g_idx[c],
            "sidx": s_idx[c],
            "fsc": fp,
            "f2sc": fp * fp,
            "embp": _pack_rows(emb_pad[c]),
        })

    res = bass_utils.run_bass_kernel_spmd(
        nc, in_maps, list(range(W)),
        trace=bool(int(os.environ.get("KERNEL_TRACE", "0"))))
    kernel.last_exec_time_ns = res.exec_time_ns

    all_emb = np.empty((N, D), np.float32)
    for c in range(W):
        o = res.results[c]["out"].reshape(128 * COLS, D)
        all_emb[c * NS:(c + 1) * NS] = o[:NS]
    return all_emb, all_emb[:NUM_USERS], all_emb[NUM_USERS:]


kernel.last_exec_time_ns = None
